# revision 1
# baseline (speedup 1.0000x reference)
"""Trainium2 Bass kernel for nn_EqPropTuned (equilibrium-propagation relaxation).

Network: DIMS = [2048, 2048, 2048, 2048, 1000], BATCH = 1024, 25 Gauss-Seidel
sweeps with lr 0.3, rho = clip(0, 1).

Sharding: data-parallel over batch across 8 cores (128 rows/core), weights
replicated. All states kept on-chip in dim-major ("transposed") layout
[dim, batch_per_core]; weight matrices streamed from HBM per sweep as
pre-tiled fp16 slabs (forward and pre-transposed backward copies). Matmuls
run in fp16 (fp32 PSUM accumulate); the master states stay fp32 on-chip,
with fp16 mirror copies feeding the PE.

Key algebraic facts used:
  - states are clipped in place, so rho() on a stored state is the identity
  - rho(x) @ W0 + b1 is constant across sweeps -> computed once at init (c1)
  - forward + backward matmul terms for one state tile accumulate into one
    PSUM group
"""

import os
import numpy as np
from contextlib import ExitStack

import concourse.bass as bass
import concourse.tile as tile
from concourse import mybir, bacc
from concourse.bass_utils import run_bass_kernel_spmd

F32 = mybir.dt.float32
F16 = mybir.dt.float16
AL = mybir.AluOpType

P = 128
DIMS = [2048, 2048, 2048, 2048, 1000]
PD = [2048, 2048, 2048, 2048, 1024]  # padded dims
KT = [d // P for d in PD]            # [16, 16, 16, 16, 8] k-tiles per dim
BATCH = 1024
N_CORES = 8
BPC = BATCH // N_CORES               # 128 batch rows per core
N_RELAX = int(os.environ.get("KERNEL_N_RELAX", "25"))
LR = 0.3

MM_DT = F16
MM_NP = np.float16


def _slab_f(W, Kp, Mp):
    """Forward slabs: out[m, p, k*P+j] = W[k*P+p, m*P+j], shape [Mp/P, P, Kp]."""
    K, M = W.shape
    Wp = np.zeros((Kp, Mp), np.float32)
    Wp[:K, :M] = W
    t = Wp.reshape(Kp // P, P, Mp // P, P)  # [k, p, m, j]
    out = np.ascontiguousarray(t.transpose(2, 1, 0, 3)).reshape(Mp // P, P, Kp)
    return out.astype(MM_NP)


def _slab_b(W, Kp, Mp):
    """Backward slabs built from W.T (contract over W's output dim)."""
    return _slab_f(np.ascontiguousarray(W.T.astype(np.float32)), Kp, Mp)


def _bias_tiles(b, Mp, scale=1.0):
    """[P, Mp/P] with out[p, m] = scale * b[m*P+p]."""
    bp = np.zeros(Mp, np.float32)
    bp[: b.shape[0]] = b * scale
    return np.ascontiguousarray(bp.reshape(Mp // P, P).T)


def build_nc():
    nc = bacc.Bacc(None, target_bir_lowering=False, debug=False)

    d_x16 = nc.declare_dram_parameter("x16T", [P, PD[0]], F16, isOutput=False)
    d_cx16 = nc.declare_dram_parameter("cx16T", [P, PD[0]], F16, isOutput=False)
    d_w = {}
    # forward slabs for W0..W3: contract over DIMS[l], output DIMS[l+1]
    for l in range(4):
        d_w[f"w{l}f"] = nc.declare_dram_parameter(
            f"w{l}f", [PD[l + 1] // P, P, PD[l]], MM_DT, isOutput=False
        )
    # backward slabs for W1..W3: contract over DIMS[l+1], output DIMS[l]
    for l in range(1, 4):
        d_w[f"w{l}b"] = nc.declare_dram_parameter(
            f"w{l}b", [PD[l] // P, P, PD[l + 1]], MM_DT, isOutput=False
        )
    d_b = {}
    for l in range(1, 5):
        d_b[f"b{l}raw"] = nc.declare_dram_parameter(
            f"b{l}raw", [P, PD[l] // P], F32, isOutput=False
        )
        d_b[f"b{l}s"] = nc.declare_dram_parameter(
            f"b{l}s", [P, PD[l] // P], F32, isOutput=False
        )
    d_out = nc.declare_dram_parameter("out", [P, PD[4]], F32, isOutput=True)

    with tile.TileContext(nc) as tc, ExitStack() as ctx:
        st = ctx.enter_context(tc.tile_pool(name="state", bufs=1))
        wp = ctx.enter_context(tc.tile_pool(name="wslab", bufs=4))
        pp = ctx.enter_context(tc.tile_pool(name="psum", bufs=8, space="PSUM"))
        tp = ctx.enter_context(tc.tile_pool(name="tmp", bufs=6))

        # persistent tensors: fp32 master states + fp16 matmul mirrors
        s = {}
        s16 = {}
        for l in range(1, 5):
            s[l] = st.tile([P, PD[l]], F32, tag=f"s{l}", name=f"s{l}")
            s16[l] = st.tile([P, PD[l]], MM_DT, tag=f"s16_{l}", name=f"s16_{l}")
        c1s = st.tile([P, PD[1]], F16, tag="c1s")
        x16 = st.tile([P, PD[0]], MM_DT, tag="x16")
        cx16 = st.tile([P, PD[0]], MM_DT, tag="cx16")
        bias = {}
        for l in range(1, 5):
            bias[f"b{l}raw"] = st.tile(
                [P, PD[l] // P], F32, tag=f"b{l}raw", name=f"b{l}raw"
            )
            bias[f"b{l}s"] = st.tile(
                [P, PD[l] // P], F32, tag=f"b{l}s", name=f"b{l}s"
            )
            nc.sync.dma_start(bias[f"b{l}raw"][:], d_b[f"b{l}raw"][:])
            nc.sync.dma_start(bias[f"b{l}s"][:], d_b[f"b{l}s"][:])

        nc.sync.dma_start(x16[:], d_x16[:])
        nc.sync.dma_start(cx16[:], d_cx16[:])

        def mm_group(psum, slab, rhs16, kt, first, last):
            for k in range(kt):
                nc.tensor.matmul(
                    psum[:],
                    slab[:, bass.ts(k, P)],
                    rhs16[:, bass.ts(k, P)],
                    start=(first and k == 0),
                    stop=(last and k == kt - 1),
                )

        # ---- init pass ----
        # layer 1 init + c1 constant share one pass over w0f
        for m in range(KT[1]):
            wf = wp.tile([P, PD[0]], MM_DT, tag="slab")
            nc.sync.dma_start(wf[:], d_w["w0f"][m])
            ps_i = pp.tile([P, P], F32, tag="ps")
            ps_c = pp.tile([P, P], F32, tag="ps")
            mm_group(ps_i, wf, x16, KT[0], True, True)
            mm_group(ps_c, wf, cx16, KT[0], True, True)
            # s1_init = clip(x @ W0 + b1)
            t = tp.tile([P, P], F32, tag="t")
            nc.vector.tensor_scalar(
                t[:], ps_i[:], bias["b1raw"][:, m : m + 1], 0.0, AL.add, AL.max
            )
            nc.vector.tensor_scalar_min(s[1][:, bass.ts(m, P)], t[:], 1.0)
            nc.gpsimd.tensor_scalar_min(s16[1][:, bass.ts(m, P)], t[:], 1.0)
            # c1s = 0.3 * (clip(x) @ W0 + b1)
            nc.vector.tensor_scalar(
                c1s[:, bass.ts(m, P)],
                ps_c[:],
                0.3,
                bias["b1s"][:, m : m + 1],
                AL.mult,
                AL.add,
            )

        # W3 (smallest matrix) stays resident in SBUF for all sweeps:
        # saves 8 MB/sweep of HBM streaming.
        w3f_res = st.tile([P, KT[4] * PD[3]], MM_DT, tag="w3f_res")
        w3b_res = st.tile([P, KT[3] * PD[4]], MM_DT, tag="w3b_res")
        for m in range(KT[4]):
            nc.sync.dma_start(
                w3f_res[:, m * PD[3] : (m + 1) * PD[3]], d_w["w3f"][m]
            )
        for m in range(KT[3]):
            nc.sync.dma_start(
                w3b_res[:, m * PD[4] : (m + 1) * PD[4]], d_w["w3b"][m]
            )
        # partial residency for W2 backward slabs (as many as SBUF allows)
        N_W2B_RES = 12
        w2b_res = st.tile([P, N_W2B_RES * PD[3]], MM_DT, tag="w2b_res")
        for m in range(N_W2B_RES):
            nc.sync.dma_start(
                w2b_res[:, m * PD[3] : (m + 1) * PD[3]], d_w["w2b"][m]
            )

        # init layers 2..4: s_{l+1} = clip(s_l @ W_l + b_{l+1})
        for l in range(1, 4):
            for m in range(KT[l + 1]):
                if l == 3:
                    wf = w3f_res[:, m * PD[3] : (m + 1) * PD[3]]
                else:
                    wf = wp.tile([P, PD[l]], MM_DT, tag="slab")
                    nc.sync.dma_start(wf[:], d_w[f"w{l}f"][m])
                ps = pp.tile([P, P], F32, tag="ps")
                mm_group(ps, wf, s16[l], KT[l], True, True)
                t = tp.tile([P, P], F32, tag="t")
                nc.vector.tensor_scalar(
                    t[:],
                    ps[:],
                    bias[f"b{l + 1}raw"][:, m : m + 1],
                    0.0,
                    AL.add,
                    AL.max,
                )
                nc.vector.tensor_scalar_min(s[l + 1][:, bass.ts(m, P)], t[:], 1.0)
                nc.gpsimd.tensor_scalar_min(
                    s16[l + 1][:, bass.ts(m, P)], t[:], 1.0
                )

        # ---- relaxation sweeps ----
        # streamed slabs are fetched in adjacent-m pairs (one 1 MB DMA instead
        # of two 0.5 MB ones) for better HBM efficiency
        for _ in range(N_RELAX):
            for l in range(1, 5):
                fwd = None if l == 1 else (d_w[f"w{l - 1}f"], s16[l - 1], KT[l - 1])
                bwd = None if l == 4 else (d_w[f"w{l}b"], s16[l + 1], KT[l + 1])
                pair_f = pair_b = None
                for m in range(KT[l]):
                    if m % 2 == 0:
                        pair_f = pair_b = None
                        if fwd is not None and l != 4:
                            kf = fwd[2] * P
                            pair_f = wp.tile([P, 2 * kf], MM_DT, tag="slab")
                            nc.sync.dma_start(
                                pair_f[:].rearrange("p (i k) -> p i k", i=2),
                                fwd[0][m : m + 2].rearrange("i p k -> p i k"),
                            )
                        if bwd is not None and l != 3 and not (
                            l == 2 and m + 1 < N_W2B_RES
                        ):
                            kb = bwd[2] * P
                            pair_b = wp.tile([P, 2 * kb], MM_DT, tag="slab")
                            nc.sync.dma_start(
                                pair_b[:].rearrange("p (i k) -> p i k", i=2),
                                bwd[0][m : m + 2].rearrange("i p k -> p i k"),
                            )
                    slabs = []
                    if fwd is not None:
                        if l == 4:
                            wf = w3f_res[:, m * PD[3] : (m + 1) * PD[3]]
                        else:
                            kf = fwd[2] * P
                            wf = pair_f[:, (m % 2) * kf : (m % 2 + 1) * kf]
                        slabs.append((wf, fwd[1], fwd[2]))
                    if bwd is not None:
                        if l == 3:
                            wb = w3b_res[:, m * PD[4] : (m + 1) * PD[4]]
                        elif l == 2 and m < N_W2B_RES:
                            wb = w2b_res[:, m * PD[3] : (m + 1) * PD[3]]
                        else:
                            kb = bwd[2] * P
                            wb = pair_b[:, (m % 2) * kb : (m % 2 + 1) * kb]
                        slabs.append((wb, bwd[1], bwd[2]))
                    ps = pp.tile([P, P], F32, tag="ps")
                    for i, (slab, rhs16, kt) in enumerate(slabs):
                        mm_group(ps, slab, rhs16, kt, i == 0, i == len(slabs) - 1)
                    # t = 0.3 * psum + 0.3 * bias   (or + 0.3 * c1 for l=1)
                    t = tp.tile([P, P], F32, tag="t")
                    if l == 1:
                        nc.vector.scalar_tensor_tensor(
                            t[:], ps[:], 0.3, c1s[:, bass.ts(m, P)], AL.mult, AL.add
                        )
                    else:
                        nc.vector.tensor_scalar(
                            t[:], ps[:], 0.3, bias[f"b{l}s"][:, m : m + 1],
                            AL.mult, AL.add,
                        )
                    # u = 0.7 * s + t ; s = clip(u, 0, 1) (fp32 + fp16 mirror)
                    u = tp.tile([P, P], F32, tag="u")
                    nc.vector.scalar_tensor_tensor(
                        u[:], s[l][:, bass.ts(m, P)], 0.7, t[:], AL.mult, AL.add
                    )
                    nc.vector.tensor_scalar(
                        s[l][:, bass.ts(m, P)], u[:], 0.0, 1.0, AL.max, AL.min
                    )
                    nc.gpsimd.tensor_scalar(
                        s16[l][:, bass.ts(m, P)], u[:], 0.0, 1.0, AL.max, AL.min
                    )

        nc.sync.dma_start(d_out[:], s[4][:])

    nc.compile()
    return nc


def _prep_inputs(x, W0, W1, W2, W3, b1, b2, b3, b4):
    """Host-side data prep shared by all cores (weights) + per-core x."""
    common = {
        "w0f": _slab_f(W0, PD[0], PD[1]),
        "w1f": _slab_f(W1, PD[1], PD[2]),
        "w2f": _slab_f(W2, PD[2], PD[3]),
        "w3f": _slab_f(W3, PD[3], PD[4]),
        "w1b": _slab_b(W1, PD[2], PD[1]),
        "w2b": _slab_b(W2, PD[3], PD[2]),
        "w3b": _slab_b(W3, PD[4], PD[3]),
    }
    for l, b in zip(range(1, 5), [b1, b2, b3, b4]):
        common[f"b{l}raw"] = _bias_tiles(b, PD[l], 1.0)
        common[f"b{l}s"] = _bias_tiles(b, PD[l], LR)

    in_maps = []
    for c in range(N_CORES):
        xs = np.asarray(x[c * BPC : (c + 1) * BPC], dtype=np.float32)
        # xT[p, k*P+j] = xs[j, k*P+p]
        xT = np.ascontiguousarray(
            xs.reshape(BPC, PD[0] // P, P).transpose(2, 1, 0)
        ).reshape(P, PD[0])
        in_maps.append({
            "x16T": xT.astype(np.float16),
            "cx16T": np.clip(xT, 0.0, 1.0).astype(np.float16),
            **common,
        })
    return in_maps


_NC_CACHE = None


def _get_nc():
    global _NC_CACHE
    if _NC_CACHE is None:
        _NC_CACHE = build_nc()
    return _NC_CACHE


def run(inputs, trace=False):
    nc = _get_nc()
    in_maps = _prep_inputs(**inputs)
    res = run_bass_kernel_spmd(nc, in_maps, list(range(N_CORES)), trace=trace)
    outs = []
    for c in range(N_CORES):
        o = res.results[c]["out"]  # [P, PD[4]] = [128, 1024]
        # decode: o[p, k*P+j] = s4T[k*P+p, j] = s4[batch j, dim k*P+p]
        s4 = o.reshape(P, PD[4] // P, P).transpose(2, 1, 0).reshape(BPC, PD[4])
        outs.append(s4[:, : DIMS[4]])
    return np.concatenate(outs, axis=0).astype(np.float32), res


def kernel(**inputs):
    out, _ = run(inputs, trace=False)
    return out



# revision 2
# speedup vs baseline: 17.2683x; 17.2683x over previous
"""Trainium2 Bass kernel for nn_EqPropTuned (equilibrium-propagation relaxation).

Network: DIMS = [2048, 2048, 2048, 2048, 1000], BATCH = 1024, 25 Gauss-Seidel
sweeps with lr 0.3, rho = clip(0, 1).

Sharding: data-parallel over batch across 8 cores (128 rows/core), weights
replicated.

Design ("state-stationary"): every matmul uses a 128x128 state tile (dim-major
fp16) as the PE stationary operand and weight rows as the moving operand (512
columns per instruction), so each stationary load is amortized over 4 wide
matmuls and the PE runs near its 16-bit roofline. Each layer accumulates into
a single 4-bank PSUM tile in batch-major layout [batch, dim]; the layer bias
is folded into the accumulation by one extra matmul against a
partition-selector stationary (out[m,j] += bias_all[l,j]). The Activation
engine then evacuates PSUM to an fp16 batch-major slab applying the 0.3
learning-rate scale (so PSUM frees ~3us after the last matmul and never gates
the next layers), the DVE stream-transposes it to dim-major fp16, and the Pool
engine applies s = clip(0.7*s + t) in-place at 512-wide chunk granularity.

Memory plan (per partition): backward (transposed) weight slabs stay resident
in SBUF (64+64+32 KB) because the backward term of layer l reads s16[l+1]
from the *previous* sweep — it has no intra-sweep dependency, so it issues
first in each layer and hides the previous layer's update tail while the
forward term's slabs stream from HBM (20 MB/sweep/core), interleaved 1:1 with
backward matmuls so the PE paces the DMA. Layer 4 (forward-only) is
interleaved with the NEXT sweep's layer 1 (backward-only). States are a
single fp16 copy (numerically validated: rel err 9.5e-3 vs 9.2e-3 with fp32
masters; fp16 staging is safe because only matmul values in (-2.4, 3.4)
escape the clip). rho() on a stored state is the identity since states are
stored post-clip; rho(x) @ W0 + b1 is constant across sweeps -> folded into
the batch-major constant c1bm at init.
"""

import os
import numpy as np
from contextlib import ExitStack

import concourse.bass as bass
import concourse.tile as tile
from concourse import mybir, bacc
from concourse.bass_utils import run_bass_kernel_spmd

F32 = mybir.dt.float32
F16 = mybir.dt.float16
AL = mybir.AluOpType
AF = mybir.ActivationFunctionType

P = 128
CH = 512                              # moving-operand / psum-bank chunk width
DIMS = [2048, 2048, 2048, 2048, 1000]
PD = [2048, 2048, 2048, 2048, 1024]   # padded dims
KT = [d // P for d in PD]             # [16, 16, 16, 16, 8] tiles per dim
BATCH = 1024
N_CORES = 8
BPC = BATCH // N_CORES                # 128 batch rows per core
N_RELAX = int(os.environ.get("KERNEL_N_RELAX", "25"))
LR = 0.3


def build_nc():
    nc = bacc.Bacc(None, target_bir_lowering=False, debug=False)

    d_x = nc.declare_dram_parameter("x16T", [P, PD[0]], F16, isOutput=False)
    d_wf = [
        nc.declare_dram_parameter(f"w{l}f", [KT[l], P, PD[l + 1]], F16, isOutput=False)
        for l in range(4)
    ]
    d_wb = {
        l: nc.declare_dram_parameter(f"w{l}b", [KT[l + 1], P, PD[l]], F16, isOutput=False)
        for l in range(1, 4)
    }
    d_ba = nc.declare_dram_parameter("bias_all", [P, PD[0]], F16, isOutput=False)
    d_em = nc.declare_dram_parameter("emat", [P, 4 * P], F16, isOutput=False)
    d_out = nc.declare_dram_parameter("out", [P, PD[4]], F16, isOutput=True)

    with tile.TileContext(nc) as tc, ExitStack() as ctx:
        st = ctx.enter_context(tc.tile_pool(name="state", bufs=1))
        wp = ctx.enter_context(tc.tile_pool(name="wstream", bufs=4))
        pp = ctx.enter_context(tc.tile_pool(name="psum", bufs=2, space="PSUM"))
        sp = ctx.enter_context(tc.tile_pool(name="stag", bufs=1))
        tp = ctx.enter_context(tc.tile_pool(name="tmp", bufs=2))

        # ---- persistent SBUF ----
        s16 = {}
        for l in range(1, 5):
            s16[l] = st.tile([P, PD[l]], F16, tag=f"s{l}", name=f"s{l}")
        c1bm = st.tile([P, PD[1]], F16, tag="c1bm")
        bias_all = st.tile([P, PD[0]], F16, tag="bias_all")
        emat = st.tile([P, 4 * P], F16, tag="emat")
        nc.sync.dma_start(bias_all[:], d_ba[:])
        nc.sync.dma_start(emat[:], d_em[:])
        # resident backward slabs: wbres[l] slab j = W_l^T[j*P+p, :PD[l]]
        wbres = {}
        for l in range(1, 4):
            wbres[l] = st.tile(
                [P, KT[l + 1] * PD[l]], F16, tag=f"wbres{l}", name=f"wbres{l}"
            )

        def bias_mm(ps, l):
            """Open the accumulation group with out[m, j] = bias_all[l, j]."""
            for c in range(PD[l] // CH):
                nc.tensor.matmul(
                    ps[:, c * CH:(c + 1) * CH],
                    emat[:, (l - 1) * P:l * P],
                    bias_all[:, c * CH:(c + 1) * CH],
                    start=True,
                    stop=False,
                )

        def dve_transpose_group(stag, bm, t0, t1):
            """stag[p, t*P + b] = bm[b, t*P + p] via 32x32 block transposes."""
            nt = t1 - t0
            stag3 = stag[:, t0 * P:t1 * P].rearrange("p (t b) -> p t b", t=nt)
            bm3 = bm[:, t0 * P:t1 * P].rearrange("b (t q) -> b t q", t=nt)
            for ai in range(4):
                for bi in range(4):
                    nc.vector.transpose(
                        stag3[ai * 32:(ai + 1) * 32, :, bi * 32:(bi + 1) * 32],
                        bm3[bi * 32:(bi + 1) * 32, :, ai * 32:(ai + 1) * 32],
                    )

        def update_layer(l, ps, kind):
            """Evacuate psum (ACT), transpose to dim-major (DVE), update state
            (Pool), all at chunk granularity so psum frees fast and early
            state tiles release before the layer's tail finishes."""
            nch = PD[l] // CH
            if kind == "c1":
                # c1bm = 0.3*(mm + b1), kept batch-major; no transpose needed
                for c in range(nch):
                    cs = slice(c * CH, (c + 1) * CH)
                    nc.scalar.activation(c1bm[:, cs], ps[:, cs], AF.Copy, scale=LR)
                return
            bm = sp.tile([P, PD[l]], F16, tag="bm", name="bm")
            for c in range(nch):
                cs = slice(c * CH, (c + 1) * CH)
                if kind == "init":
                    # s = clip(mm + b): relu here, min(.,1) after transpose
                    nc.scalar.activation(bm[:, cs], ps[:, cs], AF.Relu)
                else:
                    # t = 0.3*(mm + b)  (bias already accumulated in psum)
                    nc.scalar.activation(bm[:, cs], ps[:, cs], AF.Copy, scale=LR)
            if kind == "sweep" and l == 1:
                for c in range(nch):
                    cs = slice(c * CH, (c + 1) * CH)
                    nc.vector.tensor_tensor(bm[:, cs], bm[:, cs], c1bm[:, cs], AL.add)
            stag = sp.tile([P, PD[l]], F16, tag="stag", name="stag")
            for g0, g1 in ((0, 4), (4, 8), (8, 12), (12, 16)):
                if g0 >= KT[l]:
                    break
                g1 = min(g1, KT[l])
                dve_transpose_group(stag, bm, g0, g1)
                cs = slice(g0 * P, g1 * P)
                if kind == "init":
                    nc.gpsimd.tensor_scalar_min(s16[l][:, cs], stag[:, cs], 1.0)
                else:
                    # s = clip(0.7*s + t), in place (u on DVE, clip on Pool —
                    # the Pool ucode only has const-scalar opcodes)
                    nc.vector.scalar_tensor_tensor(
                        s16[l][:, cs], s16[l][:, cs], 1.0 - LR, stag[:, cs],
                        AL.mult, AL.add,
                    )
                    nc.gpsimd.tensor_scalar(
                        s16[l][:, cs], s16[l][:, cs], 0.0, 1.0, AL.max, AL.min
                    )

        # ---- init ----
        # layer-1 init + c1 share one streamed pass over W0
        ps_h = pp.tile([P, PD[1]], F32, tag="ps", name="ps_h")
        ps_c = pp.tile([P, PD[1]], F32, tag="ps", name="ps_c")
        bias_mm(ps_h, 1)
        bias_mm(ps_c, 1)
        for k in range(KT[0]):
            w0s = wp.tile([P, PD[1]], F16, tag="wslab", name="w0s")
            nc.sync.dma_start(w0s[:], d_wf[0][k])
            xk = tp.tile([P, P], F16, tag="xk", bufs=1, name="xk")
            nc.sync.dma_start(xk[:], d_x[:, bass.ts(k, P)])
            cxk = tp.tile([P, P], F16, tag="cxk", bufs=1, name="cxk")
            nc.vector.tensor_scalar(cxk[:], xk[:], 0.0, 1.0, AL.max, AL.min)
            last = k == KT[0] - 1
            for c in range(PD[1] // CH):
                mv = w0s[:, c * CH:(c + 1) * CH]
                nc.tensor.matmul(
                    ps_h[:, c * CH:(c + 1) * CH], xk[:], mv, start=False, stop=last
                )
                nc.tensor.matmul(
                    ps_c[:, c * CH:(c + 1) * CH], cxk[:], mv, start=False, stop=last
                )
        update_layer(1, ps_h, "init")
        update_layer(1, ps_c, "c1")

        # layers 2..4 init: s_{l+1} = clip(s_l @ W_l + b_{l+1}), streamed fwd
        # (slab DMAs issued 3 tiles ahead of their matmuls)
        for l in range(1, 4):
            ps = pp.tile([P, PD[l + 1]], F32, tag="ps", name="ps_i")
            bias_mm(ps, l + 1)
            islabs = {}
            for k in range(min(3, KT[l])):
                islabs[k] = wp.tile([P, PD[1]], F16, tag="wslab", name="wfi")
                nc.sync.dma_start(islabs[k][:, : PD[l + 1]], d_wf[l][k])
            for k in range(KT[l]):
                if k + 3 < KT[l]:
                    islabs[k + 3] = wp.tile([P, PD[1]], F16, tag="wslab", name="wfi")
                    nc.sync.dma_start(islabs[k + 3][:, : PD[l + 1]], d_wf[l][k + 3])
                slab = islabs.pop(k)
                last = k == KT[l] - 1
                for c in range(PD[l + 1] // CH):
                    nc.tensor.matmul(
                        ps[:, c * CH:(c + 1) * CH],
                        s16[l][:, bass.ts(k, P)],
                        slab[:, c * CH:(c + 1) * CH],
                        start=False,
                        stop=last,
                    )
            update_layer(l + 1, ps, "init")

        # resident backward slab loads: issued after the init streams so the
        # init pass is not stuck behind 21 MB of resident DMA; needed first by
        # sweep-0 layer 1.
        for l in range(1, 4):
            for j in range(KT[l + 1]):
                nc.sync.dma_start(
                    wbres[l][:, j * PD[l]:(j + 1) * PD[l]], d_wb[l][j]
                )

        # ---- relaxation sweeps ----
        def layer_steps(l):
            """[(l, 'b'|'f', idx), ...] in issue order for one layer."""
            nb = KT[l + 1] if l < 4 else 0
            nf = KT[l - 1] if l > 1 else 0
            bq = [(l, "b", j) for j in range(nb)]
            fq = [(l, "f", k) for k in range(nf)]
            head = bq[:6]
            bq = bq[6:]
            steps = head
            while bq or fq:
                if fq:
                    steps.append(fq.pop(0))
                if bq:
                    steps.append(bq.pop(0))
            return steps

        def emit_phase(parts):
            """Emit the matmuls for one or two layers, then the psum
            evacuations/updates in completion order."""
            if len(parts) == 1:
                steps = layer_steps(parts[0])
            else:
                a, b = parts  # next-sweep l1 (bwd-only), then l4 (fwd-only)
                steps = layer_steps(a) + layer_steps(b)
            pss = {}
            remaining = {}
            for l, _, _ in steps:
                if l not in pss:
                    pss[l] = pp.tile([P, PD[l]], F32, tag="ps", name=f"ps_l{l}")
                    remaining[l] = 0
                remaining[l] += 1
            for l in pss:
                if l > 1:
                    bias_mm(pss[l], l)
            # forward slab DMAs are issued ~3 steps ahead of their matmuls so
            # the DMA completion latency is off the PE critical path
            fwd_steps = [(l, idx) for l, k_, idx in steps if k_ == "f"]
            fwd_slabs = {}
            n_issued = 0
            n_consumed = 0

            def prefetch():
                nonlocal n_issued
                while n_issued < len(fwd_steps) and n_issued < n_consumed + 3:
                    fl, fidx = fwd_steps[n_issued]
                    slab = wp.tile([P, PD[1]], F16, tag="wslab", name="wfs")
                    nc.sync.dma_start(slab[:, : PD[fl]], d_wf[fl - 1][fidx])
                    fwd_slabs[(fl, fidx)] = slab
                    n_issued += 1

            started = set()
            order = []
            prefetch()
            for l, kind_, idx in steps:
                first = (l == 1) and (l not in started)
                started.add(l)
                remaining[l] -= 1
                last = remaining[l] == 0
                if last:
                    order.append(l)
                if kind_ == "b":
                    stat = s16[l + 1][:, bass.ts(idx, P)]
                    mvbase = wbres[l][:, idx * PD[l]:(idx + 1) * PD[l]]
                else:
                    slab = fwd_slabs.pop((l, idx))
                    stat = s16[l - 1][:, bass.ts(idx, P)]
                    mvbase = slab[:, : PD[l]]
                    n_consumed += 1
                    prefetch()
                for c in range(PD[l] // CH):
                    nc.tensor.matmul(
                        pss[l][:, c * CH:(c + 1) * CH],
                        stat,
                        mvbase[:, c * CH:(c + 1) * CH],
                        start=first,
                        stop=last,
                    )
            for l in order:
                update_layer(l, pss[l], "sweep")

        emit_phase([1])                      # sweep 0, layer 1
        for i in range(N_RELAX):
            emit_phase([2])
            emit_phase([3])
            if i + 1 < N_RELAX:
                emit_phase([1, 4])           # this l4 + next sweep's l1
            else:
                emit_phase([4])

        nc.sync.dma_start(d_out[:], s16[4][:])

    nc.compile()
    return nc


def _prep_inputs(x, W0, W1, W2, W3, b1, b2, b3, b4):
    Ws = [W0, W1, W2, W3]
    common = {}
    for l in range(4):
        Wp = np.zeros((PD[l], PD[l + 1]), np.float16)
        Wp[: Ws[l].shape[0], : Ws[l].shape[1]] = np.asarray(Ws[l], np.float32)
        common[f"w{l}f"] = Wp.reshape(KT[l], P, PD[l + 1])
    for l in range(1, 4):
        WT = np.zeros((PD[l + 1], PD[l]), np.float16)
        WT[: Ws[l].shape[1], : Ws[l].shape[0]] = np.asarray(Ws[l], np.float32).T
        common[f"w{l}b"] = WT.reshape(KT[l + 1], P, PD[l])
    ba = np.zeros((P, PD[0]), np.float16)
    for l, b in zip(range(1, 5), [b1, b2, b3, b4]):
        ba[l, : b.shape[0]] = np.asarray(b, np.float32)
    common["bias_all"] = ba
    em = np.zeros((P, 4 * P), np.float16)
    for l in range(1, 5):
        em[l, (l - 1) * P:l * P] = 1.0
    common["emat"] = em

    in_maps = []
    for c in range(N_CORES):
        xs = np.asarray(x[c * BPC:(c + 1) * BPC], dtype=np.float32)
        xT = np.ascontiguousarray(
            xs.reshape(BPC, PD[0] // P, P).transpose(2, 1, 0)
        ).reshape(P, PD[0])
        in_maps.append({"x16T": xT.astype(np.float16), **common})
    return in_maps


_NC_CACHE = None


def _get_nc():
    global _NC_CACHE
    if _NC_CACHE is None:
        _NC_CACHE = build_nc()
    return _NC_CACHE


def run(inputs, trace=False):
    nc = _get_nc()
    in_maps = _prep_inputs(**inputs)
    res = run_bass_kernel_spmd(nc, in_maps, list(range(N_CORES)), trace=trace)
    outs = []
    for c in range(N_CORES):
        o = res.results[c]["out"]  # [P, PD[4]] fp16, dim-major
        s4 = o.reshape(P, PD[4] // P, P).transpose(2, 1, 0).reshape(BPC, PD[4])
        outs.append(s4[:, : DIMS[4]].astype(np.float32))
    return np.concatenate(outs, axis=0), res


def kernel(**inputs):
    out, _ = run(inputs, trace=False)
    return out


# revision 3
# speedup vs baseline: 19.1803x; 1.1107x over previous
"""Trainium2 Bass kernel for nn_EqPropTuned (equilibrium-propagation relaxation).

Network: DIMS = [2048, 2048, 2048, 2048, 1000], BATCH = 1024, 25 Gauss-Seidel
sweeps with lr 0.3, rho = clip(0, 1).

Sharding: data-parallel over batch across 8 cores (128 rows/core), weights
replicated.

Design ("state-stationary"): every matmul uses a 128x128 state tile (dim-major
fp16) as the PE stationary operand and weight rows as the moving operand (512
columns per instruction), so each stationary load is amortized over 4 wide
matmuls and the PE runs near its 16-bit roofline. Each layer accumulates into
a single 4-bank PSUM tile in batch-major layout [batch, dim]; the layer bias
is folded into the accumulation by one extra matmul against a
partition-selector stationary (out[m,j] += bias_all[l,j]). The Activation
engine then evacuates PSUM to an fp16 batch-major slab applying the 0.3
learning-rate scale (so PSUM frees ~3us after the last matmul and never gates
the next layers), the DVE stream-transposes it to dim-major fp16, and the Pool
engine applies s = clip(0.7*s + t) in-place at 512-wide chunk granularity.

Memory plan (per partition): backward (transposed) weight slabs stay resident
in SBUF (64+64+32 KB) because the backward term of layer l reads s16[l+1]
from the *previous* sweep — it has no intra-sweep dependency, so it issues
first in each layer and hides the previous layer's update tail while the
forward term's slabs stream from HBM (20 MB/sweep/core), interleaved 1:1 with
backward matmuls so the PE paces the DMA. Layer 4 (forward-only) is
interleaved with the NEXT sweep's layer 1 (backward-only). States are a
single fp16 copy (numerically validated: rel err 9.5e-3 vs 9.2e-3 with fp32
masters; fp16 staging is safe because only matmul values in (-2.4, 3.4)
escape the clip). rho() on a stored state is the identity since states are
stored post-clip; rho(x) @ W0 + b1 is constant across sweeps -> folded into
the batch-major constant c1bm at init.
"""

import os
import numpy as np
from contextlib import ExitStack

import concourse.bass as bass
import concourse.tile as tile
from concourse import mybir, bacc
from concourse.bass_utils import run_bass_kernel_spmd

F32 = mybir.dt.float32
F16 = mybir.dt.float16
AL = mybir.AluOpType
AF = mybir.ActivationFunctionType

P = 128
CH = 512                              # moving-operand / psum-bank chunk width
DIMS = [2048, 2048, 2048, 2048, 1000]
PD = [2048, 2048, 2048, 2048, 1024]   # padded dims
KT = [d // P for d in PD]             # [16, 16, 16, 16, 8] tiles per dim
BATCH = 1024
N_CORES = 8
BPC = BATCH // N_CORES                # 128 batch rows per core
N_RELAX = int(os.environ.get("KERNEL_N_RELAX", "25"))
LR = 0.3


def build_nc():
    nc = bacc.Bacc(None, target_bir_lowering=False, debug=False)

    d_x = nc.declare_dram_parameter("x16T", [P, PD[0]], F16, isOutput=False)
    d_wf = [
        nc.declare_dram_parameter(f"w{l}f", [KT[l], P, PD[l + 1]], F16, isOutput=False)
        for l in range(4)
    ]
    d_wb = {
        l: nc.declare_dram_parameter(f"w{l}b", [KT[l + 1], P, PD[l]], F16, isOutput=False)
        for l in range(1, 4)
    }
    d_ba = nc.declare_dram_parameter("bias_all", [P, PD[0]], F16, isOutput=False)
    d_em = nc.declare_dram_parameter("emat", [P, 4 * P], F16, isOutput=False)
    d_out = nc.declare_dram_parameter("out", [P, PD[4]], F16, isOutput=True)

    with tile.TileContext(nc) as tc, ExitStack() as ctx:
        st = ctx.enter_context(tc.tile_pool(name="state", bufs=1))
        wp = ctx.enter_context(tc.tile_pool(name="wstream", bufs=8))
        pp = ctx.enter_context(tc.tile_pool(name="psum", bufs=2, space="PSUM"))
        sp = ctx.enter_context(tc.tile_pool(name="stag", bufs=1))
        tp = ctx.enter_context(tc.tile_pool(name="tmp", bufs=2))

        # ---- persistent SBUF ----
        s16 = {}
        for l in range(1, 5):
            s16[l] = st.tile([P, PD[l]], F16, tag=f"s{l}", name=f"s{l}")
        c1bm = st.tile([P, PD[1]], F16, tag="c1bm")
        bias_all = st.tile([P, PD[0]], F16, tag="bias_all")
        emat = st.tile([P, 4 * P], F16, tag="emat")
        nc.sync.dma_start(bias_all[:], d_ba[:])
        nc.sync.dma_start(emat[:], d_em[:])
        # resident backward slabs: wbres[l] slab j = W_l^T[j*P+p, :PD[l]]
        wbres = {}
        for l in range(1, 4):
            wbres[l] = st.tile(
                [P, KT[l + 1] * PD[l]], F16, tag=f"wbres{l}", name=f"wbres{l}"
            )

        def bias_mm(ps, l):
            """Open the accumulation group with out[m, j] = bias_all[l, j]."""
            for c in range(PD[l] // CH):
                nc.tensor.matmul(
                    ps[:, c * CH:(c + 1) * CH],
                    emat[:, (l - 1) * P:l * P],
                    bias_all[:, c * CH:(c + 1) * CH],
                    start=True,
                    stop=False,
                )

        def dve_transpose_group(stag, bm, t0, t1):
            """stag[p, t*P + b] = bm[b, t*P + p] via 32x32 block transposes."""
            nt = t1 - t0
            stag3 = stag[:, t0 * P:t1 * P].rearrange("p (t b) -> p t b", t=nt)
            bm3 = bm[:, t0 * P:t1 * P].rearrange("b (t q) -> b t q", t=nt)
            for ai in range(4):
                for bi in range(4):
                    nc.vector.transpose(
                        stag3[ai * 32:(ai + 1) * 32, :, bi * 32:(bi + 1) * 32],
                        bm3[bi * 32:(bi + 1) * 32, :, ai * 32:(ai + 1) * 32],
                    )

        def update_layer(l, ps, kind):
            """Evacuate psum (ACT), transpose to dim-major (DVE), update state
            (Pool), all at chunk granularity so psum frees fast and early
            state tiles release before the layer's tail finishes."""
            nch = PD[l] // CH
            if kind == "c1":
                # c1bm = 0.3*(mm + b1), kept batch-major; no transpose needed
                for c in range(nch):
                    cs = slice(c * CH, (c + 1) * CH)
                    nc.scalar.activation(c1bm[:, cs], ps[:, cs], AF.Copy, scale=LR)
                return
            bm = sp.tile([P, PD[l]], F16, tag="bm", name="bm")
            for c in range(nch):
                cs = slice(c * CH, (c + 1) * CH)
                if kind == "init":
                    # s = clip(mm + b): relu here, min(.,1) after transpose
                    nc.scalar.activation(bm[:, cs], ps[:, cs], AF.Relu)
                else:
                    # t = 0.3*(mm + b)  (bias already accumulated in psum)
                    nc.scalar.activation(bm[:, cs], ps[:, cs], AF.Copy, scale=LR)
            if kind == "sweep" and l == 1:
                for c in range(nch):
                    cs = slice(c * CH, (c + 1) * CH)
                    nc.vector.tensor_tensor(bm[:, cs], bm[:, cs], c1bm[:, cs], AL.add)
            stag = sp.tile([P, PD[l]], F16, tag="stag", name="stag")
            for g0, g1 in ((0, 4), (4, 8), (8, 12), (12, 16)):
                if g0 >= KT[l]:
                    break
                g1 = min(g1, KT[l])
                dve_transpose_group(stag, bm, g0, g1)
                cs = slice(g0 * P, g1 * P)
                if kind == "init":
                    nc.gpsimd.tensor_scalar_min(s16[l][:, cs], stag[:, cs], 1.0)
                else:
                    # s = clip(0.7*s + t), in place (u on DVE, clip on Pool —
                    # the Pool ucode only has const-scalar opcodes)
                    nc.vector.scalar_tensor_tensor(
                        s16[l][:, cs], s16[l][:, cs], 1.0 - LR, stag[:, cs],
                        AL.mult, AL.add,
                    )
                    nc.gpsimd.tensor_scalar(
                        s16[l][:, cs], s16[l][:, cs], 0.0, 1.0, AL.max, AL.min
                    )

        # ---- init ----
        # layer-1 init + c1 share one streamed pass over W0
        ps_h = pp.tile([P, PD[1]], F32, tag="ps", name="ps_h")
        ps_c = pp.tile([P, PD[1]], F32, tag="ps", name="ps_c")
        bias_mm(ps_h, 1)
        bias_mm(ps_c, 1)
        H0 = PD[1] // 2
        w0h = {}
        pend0 = [(k, h) for k in range(KT[0]) for h in range(2)]

        def load_w0():
            k, h = pend0.pop(0)
            t = wp.tile([P, H0], F16, tag="wslab", name="w0s")
            nc.sync.dma_start(t[:], d_wf[0][k][:, h * H0:(h + 1) * H0])
            w0h[(k, h)] = t

        for _ in range(6):
            load_w0()
        for k in range(KT[0]):
            xk = tp.tile([P, P], F16, tag="xk", bufs=1, name="xk")
            nc.sync.dma_start(xk[:], d_x[:, bass.ts(k, P)])
            cxk = tp.tile([P, P], F16, tag="cxk", bufs=1, name="cxk")
            nc.vector.tensor_scalar(cxk[:], xk[:], 0.0, 1.0, AL.max, AL.min)
            last = k == KT[0] - 1
            for c in range(PD[1] // CH):
                h = (c * CH) // H0
                if (c * CH) % H0 == 0:
                    if pend0:
                        load_w0()
                    half = w0h.pop((k, h))
                off = (c * CH) % H0
                mv = half[:, off:off + CH]
                nc.tensor.matmul(
                    ps_h[:, c * CH:(c + 1) * CH], xk[:], mv, start=False, stop=last
                )
                nc.tensor.matmul(
                    ps_c[:, c * CH:(c + 1) * CH], cxk[:], mv, start=False, stop=last
                )
        update_layer(1, ps_h, "init")
        update_layer(1, ps_c, "c1")

        # layers 2..4 init: s_{l+1} = clip(s_l @ W_l + b_{l+1}), streamed fwd
        # (slab DMAs issued 3 tiles ahead of their matmuls)
        for l in range(1, 4):
            ps = pp.tile([P, PD[l + 1]], F32, tag="ps", name="ps_i")
            bias_mm(ps, l + 1)
            H = PD[1] // 2

            def load_half(k, h, w):
                t = wp.tile([P, H], F16, tag="wslab", name="wfi")
                nc.sync.dma_start(t[:, : min(H, w - h * H)],
                                  d_wf[l][k][:, h * H:min((h + 1) * H, w)])
                return t

            width = PD[l + 1]
            nh = (width + H - 1) // H
            ihalves = {}
            pend = [(k, h) for k in range(KT[l]) for h in range(nh)]
            for _ in range(min(6, len(pend))):
                k, h = pend.pop(0)
                ihalves[(k, h)] = load_half(k, h, width)
            for k in range(KT[l]):
                last = k == KT[l] - 1
                for c in range(width // CH):
                    h = (c * CH) // H
                    if pend:
                        k2, h2 = pend.pop(0)
                        ihalves[(k2, h2)] = load_half(k2, h2, width)
                    half = ihalves[(k, h)]
                    nc.tensor.matmul(
                        ps[:, c * CH:(c + 1) * CH],
                        s16[l][:, bass.ts(k, P)],
                        half[:, (c * CH) % H:(c * CH) % H + CH],
                        start=False,
                        stop=last and c == width // CH - 1,
                    )
            update_layer(l + 1, ps, "init")

        # resident backward slab loads: issued after the init streams so the
        # init pass is not stuck behind 21 MB of resident DMA; needed first by
        # sweep-0 layer 1.
        for l in range(1, 4):
            for j in range(KT[l + 1]):
                nc.sync.dma_start(
                    wbres[l][:, j * PD[l]:(j + 1) * PD[l]], d_wb[l][j]
                )

        # ---- relaxation sweeps ----
        def layer_steps(l):
            """[(l, 'b'|'f', idx), ...] in issue order for one layer."""
            nb = KT[l + 1] if l < 4 else 0
            nf = KT[l - 1] if l > 1 else 0
            bq = [(l, "b", j) for j in range(nb)]
            fq = [(l, "f", k) for k in range(nf)]
            head = bq[:6]
            bq = bq[6:]
            steps = head
            while bq or fq:
                if fq:
                    steps.append(fq.pop(0))
                if bq:
                    steps.append(bq.pop(0))
            return steps

        def emit_phase(parts):
            """Emit the matmuls for one or two layers, then the psum
            evacuations/updates in completion order."""
            if len(parts) == 1:
                steps = layer_steps(parts[0])
            else:
                a, b = parts  # next-sweep l1 (bwd-only), then l4 (fwd-only)
                steps = layer_steps(a) + layer_steps(b)
            pss = {}
            remaining = {}
            for l, _, _ in steps:
                if l not in pss:
                    pss[l] = pp.tile([P, PD[l]], F32, tag="ps", name=f"ps_l{l}")
                    remaining[l] = 0
                remaining[l] += 1
            for l in pss:
                if l > 1:
                    bias_mm(pss[l], l)
            # forward slab DMAs are issued ~3 steps ahead of their matmuls so
            # the DMA completion latency is off the PE critical path
            H = PD[1] // 2
            fwd_halves = []
            for l, k_, idx in steps:
                if k_ == "f":
                    for h in range((PD[l] + H - 1) // H):
                        fwd_halves.append((l, idx, h))
            fwd_slabs = {}
            n_issued = 0
            n_consumed = 0

            def prefetch():
                nonlocal n_issued
                while n_issued < len(fwd_halves) and n_issued < n_consumed + 6:
                    fl, fidx, fh = fwd_halves[n_issued]
                    w = min(H, PD[fl] - fh * H)
                    t = wp.tile([P, H], F16, tag="wslab", name="wfs")
                    nc.sync.dma_start(t[:, :w], d_wf[fl - 1][fidx][:, fh * H:fh * H + w])
                    fwd_slabs[(fl, fidx, fh)] = t
                    n_issued += 1

            started = set()
            order = []
            prefetch()
            for l, kind_, idx in steps:
                first = (l == 1) and (l not in started)
                started.add(l)
                remaining[l] -= 1
                last = remaining[l] == 0
                if last:
                    order.append(l)
                if kind_ == "b":
                    stat = s16[l + 1][:, bass.ts(idx, P)]
                    for c in range(PD[l] // CH):
                        nc.tensor.matmul(
                            pss[l][:, c * CH:(c + 1) * CH],
                            stat,
                            wbres[l][:, idx * PD[l] + c * CH:idx * PD[l] + (c + 1) * CH],
                            start=first,
                            stop=last,
                        )
                else:
                    stat = s16[l - 1][:, bass.ts(idx, P)]
                    for c in range(PD[l] // CH):
                        h = (c * CH) // H
                        if (c * CH) % H == 0:
                            half = fwd_slabs.pop((l, idx, h))
                            n_consumed += 1
                            prefetch()
                        nc.tensor.matmul(
                            pss[l][:, c * CH:(c + 1) * CH],
                            stat,
                            half[:, (c * CH) % H:(c * CH) % H + CH],
                            start=first,
                            stop=last,
                        )
            for l in order:
                update_layer(l, pss[l], "sweep")

        emit_phase([1])                      # sweep 0, layer 1
        for i in range(N_RELAX):
            emit_phase([2])
            emit_phase([3])
            if i + 1 < N_RELAX:
                emit_phase([1, 4])           # this l4 + next sweep's l1
            else:
                emit_phase([4])

        nc.sync.dma_start(d_out[:], s16[4][:])

    nc.compile()
    return nc


def _prep_inputs(x, W0, W1, W2, W3, b1, b2, b3, b4):
    Ws = [W0, W1, W2, W3]
    common = {}
    for l in range(4):
        Wp = np.zeros((PD[l], PD[l + 1]), np.float16)
        Wp[: Ws[l].shape[0], : Ws[l].shape[1]] = np.asarray(Ws[l], np.float32)
        common[f"w{l}f"] = Wp.reshape(KT[l], P, PD[l + 1])
    for l in range(1, 4):
        WT = np.zeros((PD[l + 1], PD[l]), np.float16)
        WT[: Ws[l].shape[1], : Ws[l].shape[0]] = np.asarray(Ws[l], np.float32).T
        common[f"w{l}b"] = WT.reshape(KT[l + 1], P, PD[l])
    ba = np.zeros((P, PD[0]), np.float16)
    for l, b in zip(range(1, 5), [b1, b2, b3, b4]):
        ba[l, : b.shape[0]] = np.asarray(b, np.float32)
    common["bias_all"] = ba
    em = np.zeros((P, 4 * P), np.float16)
    for l in range(1, 5):
        em[l, (l - 1) * P:l * P] = 1.0
    common["emat"] = em

    in_maps = []
    for c in range(N_CORES):
        xs = np.asarray(x[c * BPC:(c + 1) * BPC], dtype=np.float32)
        xT = np.ascontiguousarray(
            xs.reshape(BPC, PD[0] // P, P).transpose(2, 1, 0)
        ).reshape(P, PD[0])
        in_maps.append({"x16T": xT.astype(np.float16), **common})
    return in_maps


_NC_CACHE = None


def _get_nc():
    global _NC_CACHE
    if _NC_CACHE is None:
        _NC_CACHE = build_nc()
    return _NC_CACHE


def run(inputs, trace=False):
    nc = _get_nc()
    in_maps = _prep_inputs(**inputs)
    res = run_bass_kernel_spmd(nc, in_maps, list(range(N_CORES)), trace=trace)
    outs = []
    for c in range(N_CORES):
        o = res.results[c]["out"]  # [P, PD[4]] fp16, dim-major
        s4 = o.reshape(P, PD[4] // P, P).transpose(2, 1, 0).reshape(BPC, PD[4])
        outs.append(s4[:, : DIMS[4]].astype(np.float32))
    return np.concatenate(outs, axis=0), res


def kernel(**inputs):
    out, _ = run(inputs, trace=False)
    return out


# revision 4
# speedup vs baseline: 19.2403x; 1.0031x over previous
"""Trainium2 Bass kernel for nn_EqPropTuned (equilibrium-propagation relaxation).

Network: DIMS = [2048, 2048, 2048, 2048, 1000], BATCH = 1024, 25 Gauss-Seidel
sweeps with lr 0.3, rho = clip(0, 1).

Sharding: data-parallel over batch across 8 cores (128 rows/core), weights
replicated.

Design ("state-stationary"): every matmul uses a 128x128 state tile (dim-major
fp16) as the PE stationary operand and weight rows as the moving operand (512
columns per instruction), so each stationary load is amortized over 4 wide
matmuls and the PE runs near its 16-bit roofline. Each layer accumulates into
a single 4-bank PSUM tile in batch-major layout [batch, dim]; the layer bias
is folded into the accumulation by one extra matmul against a
partition-selector stationary (out[m,j] += bias_all[l,j]). The Activation
engine then evacuates PSUM to an fp16 batch-major slab applying the 0.3
learning-rate scale (so PSUM frees ~3us after the last matmul and never gates
the next layers), the DVE stream-transposes it to dim-major fp16, and the Pool
engine applies s = clip(0.7*s + t) in-place at 512-wide chunk granularity.

Memory plan (per partition): backward (transposed) weight slabs stay resident
in SBUF (64+64+32 KB) because the backward term of layer l reads s16[l+1]
from the *previous* sweep — it has no intra-sweep dependency, so it issues
first in each layer and hides the previous layer's update tail while the
forward term's slabs stream from HBM (20 MB/sweep/core), interleaved 1:1 with
backward matmuls so the PE paces the DMA. Layer 4 (forward-only) is
interleaved with the NEXT sweep's layer 1 (backward-only). States are a
single fp16 copy (numerically validated: rel err 9.5e-3 vs 9.2e-3 with fp32
masters; fp16 staging is safe because only matmul values in (-2.4, 3.4)
escape the clip). rho() on a stored state is the identity since states are
stored post-clip; rho(x) @ W0 + b1 is constant across sweeps -> folded into
the batch-major constant c1bm at init.
"""

import os
import numpy as np
from contextlib import ExitStack

import concourse.bass as bass
import concourse.tile as tile
from concourse import mybir, bacc
from concourse.bass_utils import run_bass_kernel_spmd

F32 = mybir.dt.float32
F16 = mybir.dt.float16
AL = mybir.AluOpType
AF = mybir.ActivationFunctionType

P = 128
CH = 512                              # moving-operand / psum-bank chunk width
DIMS = [2048, 2048, 2048, 2048, 1000]
PD = [2048, 2048, 2048, 2048, 1024]   # padded dims
KT = [d // P for d in PD]             # [16, 16, 16, 16, 8] tiles per dim
BATCH = 1024
N_CORES = 8
BPC = BATCH // N_CORES                # 128 batch rows per core
N_RELAX = int(os.environ.get("KERNEL_N_RELAX", "25"))
LR = 0.3


def build_nc():
    nc = bacc.Bacc(None, target_bir_lowering=False, debug=False)

    d_x = nc.declare_dram_parameter("x16T", [P, PD[0]], F16, isOutput=False)
    d_wf = [
        nc.declare_dram_parameter(f"w{l}f", [KT[l], P, PD[l + 1]], F16, isOutput=False)
        for l in range(4)
    ]
    d_wb = {
        l: nc.declare_dram_parameter(f"w{l}b", [KT[l + 1], P, PD[l]], F16, isOutput=False)
        for l in range(1, 4)
    }
    d_ba = nc.declare_dram_parameter("bias_all", [P, PD[0]], F16, isOutput=False)
    d_em = nc.declare_dram_parameter("emat", [P, 4 * P], F16, isOutput=False)
    d_out = nc.declare_dram_parameter("out", [P, PD[4]], F16, isOutput=True)

    with tile.TileContext(nc) as tc, ExitStack() as ctx:
        st = ctx.enter_context(tc.tile_pool(name="state", bufs=1))
        wp = ctx.enter_context(tc.tile_pool(name="wstream", bufs=8))
        pp = ctx.enter_context(tc.tile_pool(name="psum", bufs=2, space="PSUM"))
        sp = ctx.enter_context(tc.tile_pool(name="stag", bufs=1))
        tp = ctx.enter_context(tc.tile_pool(name="tmp", bufs=2))

        # ---- persistent SBUF ----
        s16 = {}
        for l in range(1, 5):
            s16[l] = st.tile([P, PD[l]], F16, tag=f"s{l}", name=f"s{l}")
        c1bm = st.tile([P, PD[1]], F16, tag="c1bm")
        bias_all = st.tile([P, PD[0]], F16, tag="bias_all")
        emat = st.tile([P, 4 * P], F16, tag="emat")
        nc.sync.dma_start(bias_all[:], d_ba[:])
        nc.sync.dma_start(emat[:], d_em[:])
        # resident backward slabs: wbres[l] slab j = W_l^T[j*P+p, :PD[l]]
        wbres = {}
        for l in range(1, 4):
            wbres[l] = st.tile(
                [P, KT[l + 1] * PD[l]], F16, tag=f"wbres{l}", name=f"wbres{l}"
            )

        def bias_mm(ps, l):
            """Open the accumulation group with out[m, j] = bias_all[l, j]."""
            for c in range(PD[l] // CH):
                nc.tensor.matmul(
                    ps[:, c * CH:(c + 1) * CH],
                    emat[:, (l - 1) * P:l * P],
                    bias_all[:, c * CH:(c + 1) * CH],
                    start=True,
                    stop=False,
                )

        def dve_transpose_group(stag, bm, t0, t1):
            """stag[p, t*P + b] = bm[b, t*P + p] via 32x32 block transposes."""
            nt = t1 - t0
            stag3 = stag[:, t0 * P:t1 * P].rearrange("p (t b) -> p t b", t=nt)
            bm3 = bm[:, t0 * P:t1 * P].rearrange("b (t q) -> b t q", t=nt)
            for ai in range(4):
                for bi in range(4):
                    nc.vector.transpose(
                        stag3[ai * 32:(ai + 1) * 32, :, bi * 32:(bi + 1) * 32],
                        bm3[bi * 32:(bi + 1) * 32, :, ai * 32:(ai + 1) * 32],
                    )

        def update_layer(l, ps, kind):
            """Evacuate psum (ACT), transpose to dim-major (DVE), update state
            (Pool), all at chunk granularity so psum frees fast and early
            state tiles release before the layer's tail finishes."""
            nch = PD[l] // CH
            if kind == "c1":
                # c1bm = 0.3*(mm + b1), kept batch-major; no transpose needed
                for c in range(nch):
                    cs = slice(c * CH, (c + 1) * CH)
                    nc.scalar.activation(c1bm[:, cs], ps[:, cs], AF.Copy, scale=LR)
                return
            # bm/stag are double-buffered at HALF-layer granularity (same
            # total bytes as one full-width buffer): the next phase's psum
            # evacuation of half 0 overlaps this phase's transposes of half 1,
            # breaking the evac->transpose cross-phase chain that stalled the
            # PE 3.5us at each phase boundary.
            HB = 2 * CH
            for hf in range((PD[l] + HB - 1) // HB):
                bmh = sp.tile([P, HB], F16, tag="bm", bufs=2, name="bmh")
                for c in range(2 * hf, 2 * hf + 2):
                    cs = slice(c * CH, (c + 1) * CH)
                    lo = slice((c * CH) % HB, (c * CH) % HB + CH)
                    if kind == "init":
                        # s = clip(mm + b): relu now, min(.,1) after transpose
                        nc.scalar.activation(bmh[:, lo], ps[:, cs], AF.Relu)
                    else:
                        # t = 0.3*(mm + b)  (bias already accumulated in psum)
                        nc.scalar.activation(bmh[:, lo], ps[:, cs], AF.Copy, scale=LR)
                    if kind == "sweep" and l == 1:
                        nc.vector.tensor_tensor(
                            bmh[:, lo], bmh[:, lo], c1bm[:, cs], AL.add
                        )
                sth = sp.tile([P, HB], F16, tag="stag", bufs=2, name="sth")
                for gg in range(2):
                    t0 = hf * 8 + gg * 4          # global tile base
                    if t0 >= KT[l]:
                        break
                    dve_transpose_group(sth, bmh, gg * 4, gg * 4 + 4)
                    cs = slice(t0 * P, (t0 + 4) * P)
                    ls = slice(gg * 4 * P, (gg * 4 + 4) * P)
                    if kind == "init":
                        nc.gpsimd.tensor_scalar_min(s16[l][:, cs], sth[:, ls], 1.0)
                    else:
                        # s = clip(0.7*s + t), in place (u on DVE, clip on
                        # Pool — the Pool ucode only has const-scalar opcodes)
                        nc.vector.scalar_tensor_tensor(
                            s16[l][:, cs], s16[l][:, cs], 1.0 - LR, sth[:, ls],
                            AL.mult, AL.add,
                        )
                        nc.gpsimd.tensor_scalar(
                            s16[l][:, cs], s16[l][:, cs], 0.0, 1.0, AL.max, AL.min
                        )

        # ---- init ----
        # layer-1 init + c1 share one streamed pass over W0
        ps_h = pp.tile([P, PD[1]], F32, tag="ps", name="ps_h")
        ps_c = pp.tile([P, PD[1]], F32, tag="ps", name="ps_c")
        bias_mm(ps_h, 1)
        bias_mm(ps_c, 1)
        H0 = PD[1] // 2
        w0h = {}
        pend0 = [(k, h) for k in range(KT[0]) for h in range(2)]

        def load_w0():
            k, h = pend0.pop(0)
            t = wp.tile([P, H0], F16, tag="wslab", name="w0s")
            nc.sync.dma_start(t[:], d_wf[0][k][:, h * H0:(h + 1) * H0])
            w0h[(k, h)] = t

        for _ in range(6):
            load_w0()
        for k in range(KT[0]):
            xk = tp.tile([P, P], F16, tag="xk", bufs=1, name="xk")
            nc.sync.dma_start(xk[:], d_x[:, bass.ts(k, P)])
            cxk = tp.tile([P, P], F16, tag="cxk", bufs=1, name="cxk")
            nc.vector.tensor_scalar(cxk[:], xk[:], 0.0, 1.0, AL.max, AL.min)
            last = k == KT[0] - 1
            for c in range(PD[1] // CH):
                h = (c * CH) // H0
                if (c * CH) % H0 == 0:
                    if pend0:
                        load_w0()
                    half = w0h.pop((k, h))
                off = (c * CH) % H0
                mv = half[:, off:off + CH]
                nc.tensor.matmul(
                    ps_h[:, c * CH:(c + 1) * CH], xk[:], mv, start=False, stop=last
                )
                nc.tensor.matmul(
                    ps_c[:, c * CH:(c + 1) * CH], cxk[:], mv, start=False, stop=last
                )
        update_layer(1, ps_h, "init")
        update_layer(1, ps_c, "c1")

        # layers 2..4 init: s_{l+1} = clip(s_l @ W_l + b_{l+1}), streamed fwd
        # (slab DMAs issued 3 tiles ahead of their matmuls)
        for l in range(1, 4):
            ps = pp.tile([P, PD[l + 1]], F32, tag="ps", name="ps_i")
            bias_mm(ps, l + 1)
            H = PD[1] // 2

            def load_half(k, h, w):
                t = wp.tile([P, H], F16, tag="wslab", name="wfi")
                nc.sync.dma_start(t[:, : min(H, w - h * H)],
                                  d_wf[l][k][:, h * H:min((h + 1) * H, w)])
                return t

            width = PD[l + 1]
            nh = (width + H - 1) // H
            ihalves = {}
            pend = [(k, h) for k in range(KT[l]) for h in range(nh)]
            for _ in range(min(6, len(pend))):
                k, h = pend.pop(0)
                ihalves[(k, h)] = load_half(k, h, width)
            for k in range(KT[l]):
                last = k == KT[l] - 1
                for c in range(width // CH):
                    h = (c * CH) // H
                    if pend:
                        k2, h2 = pend.pop(0)
                        ihalves[(k2, h2)] = load_half(k2, h2, width)
                    half = ihalves[(k, h)]
                    nc.tensor.matmul(
                        ps[:, c * CH:(c + 1) * CH],
                        s16[l][:, bass.ts(k, P)],
                        half[:, (c * CH) % H:(c * CH) % H + CH],
                        start=False,
                        stop=last and c == width // CH - 1,
                    )
            update_layer(l + 1, ps, "init")

        # resident backward slab loads: issued after the init streams so the
        # init pass is not stuck behind 21 MB of resident DMA; needed first by
        # sweep-0 layer 1.
        for l in range(1, 4):
            for j in range(KT[l + 1]):
                nc.sync.dma_start(
                    wbres[l][:, j * PD[l]:(j + 1) * PD[l]], d_wb[l][j]
                )

        # ---- relaxation sweeps ----
        def layer_steps(l):
            """[(l, 'b'|'f', idx), ...] in issue order for one layer."""
            nb = KT[l + 1] if l < 4 else 0
            nf = KT[l - 1] if l > 1 else 0
            bq = [(l, "b", j) for j in range(nb)]
            fq = [(l, "f", k) for k in range(nf)]
            head = bq[:6]
            bq = bq[6:]
            steps = head
            while bq or fq:
                if fq:
                    steps.append(fq.pop(0))
                if bq:
                    steps.append(bq.pop(0))
            return steps

        def emit_phase(parts):
            """Emit the matmuls for one or two layers, then the psum
            evacuations/updates in completion order."""
            if len(parts) == 1:
                steps = layer_steps(parts[0])
            else:
                a, b = parts  # next-sweep l1 (bwd-only), then l4 (fwd-only)
                steps = layer_steps(a) + layer_steps(b)
            pss = {}
            remaining = {}
            for l, _, _ in steps:
                if l not in pss:
                    pss[l] = pp.tile([P, PD[l]], F32, tag="ps", name=f"ps_l{l}")
                    remaining[l] = 0
                remaining[l] += 1
            for l in pss:
                if l > 1:
                    bias_mm(pss[l], l)
            # forward slab DMAs are issued ~3 steps ahead of their matmuls so
            # the DMA completion latency is off the PE critical path
            H = PD[1] // 2
            fwd_halves = []
            for l, k_, idx in steps:
                if k_ == "f":
                    for h in range((PD[l] + H - 1) // H):
                        fwd_halves.append((l, idx, h))
            fwd_slabs = {}
            n_issued = 0
            n_consumed = 0

            def prefetch():
                nonlocal n_issued
                while n_issued < len(fwd_halves) and n_issued < n_consumed + 6:
                    fl, fidx, fh = fwd_halves[n_issued]
                    w = min(H, PD[fl] - fh * H)
                    t = wp.tile([P, H], F16, tag="wslab", name="wfs")
                    nc.sync.dma_start(t[:, :w], d_wf[fl - 1][fidx][:, fh * H:fh * H + w])
                    fwd_slabs[(fl, fidx, fh)] = t
                    n_issued += 1

            started = set()
            order = []
            prefetch()
            for l, kind_, idx in steps:
                first = (l == 1) and (l not in started)
                started.add(l)
                remaining[l] -= 1
                last = remaining[l] == 0
                if last:
                    order.append(l)
                if kind_ == "b":
                    stat = s16[l + 1][:, bass.ts(idx, P)]
                    for c in range(PD[l] // CH):
                        nc.tensor.matmul(
                            pss[l][:, c * CH:(c + 1) * CH],
                            stat,
                            wbres[l][:, idx * PD[l] + c * CH:idx * PD[l] + (c + 1) * CH],
                            start=first,
                            stop=last,
                        )
                else:
                    stat = s16[l - 1][:, bass.ts(idx, P)]
                    for c in range(PD[l] // CH):
                        h = (c * CH) // H
                        if (c * CH) % H == 0:
                            half = fwd_slabs.pop((l, idx, h))
                            n_consumed += 1
                            prefetch()
                        nc.tensor.matmul(
                            pss[l][:, c * CH:(c + 1) * CH],
                            stat,
                            half[:, (c * CH) % H:(c * CH) % H + CH],
                            start=first,
                            stop=last,
                        )
            for l in order:
                update_layer(l, pss[l], "sweep")

        emit_phase([1])                      # sweep 0, layer 1
        for i in range(N_RELAX):
            emit_phase([2])
            emit_phase([3])
            if i + 1 < N_RELAX:
                emit_phase([1, 4])           # this l4 + next sweep's l1
            else:
                emit_phase([4])

        nc.sync.dma_start(d_out[:], s16[4][:])

    nc.compile()
    return nc


def _prep_inputs(x, W0, W1, W2, W3, b1, b2, b3, b4):
    Ws = [W0, W1, W2, W3]
    common = {}
    for l in range(4):
        Wp = np.zeros((PD[l], PD[l + 1]), np.float16)
        Wp[: Ws[l].shape[0], : Ws[l].shape[1]] = np.asarray(Ws[l], np.float32)
        common[f"w{l}f"] = Wp.reshape(KT[l], P, PD[l + 1])
    for l in range(1, 4):
        WT = np.zeros((PD[l + 1], PD[l]), np.float16)
        WT[: Ws[l].shape[1], : Ws[l].shape[0]] = np.asarray(Ws[l], np.float32).T
        common[f"w{l}b"] = WT.reshape(KT[l + 1], P, PD[l])
    ba = np.zeros((P, PD[0]), np.float16)
    for l, b in zip(range(1, 5), [b1, b2, b3, b4]):
        ba[l, : b.shape[0]] = np.asarray(b, np.float32)
    common["bias_all"] = ba
    em = np.zeros((P, 4 * P), np.float16)
    for l in range(1, 5):
        em[l, (l - 1) * P:l * P] = 1.0
    common["emat"] = em

    in_maps = []
    for c in range(N_CORES):
        xs = np.asarray(x[c * BPC:(c + 1) * BPC], dtype=np.float32)
        xT = np.ascontiguousarray(
            xs.reshape(BPC, PD[0] // P, P).transpose(2, 1, 0)
        ).reshape(P, PD[0])
        in_maps.append({"x16T": xT.astype(np.float16), **common})
    return in_maps


_NC_CACHE = None


def _get_nc():
    global _NC_CACHE
    if _NC_CACHE is None:
        _NC_CACHE = build_nc()
    return _NC_CACHE


def run(inputs, trace=False):
    nc = _get_nc()
    in_maps = _prep_inputs(**inputs)
    res = run_bass_kernel_spmd(nc, in_maps, list(range(N_CORES)), trace=trace)
    outs = []
    for c in range(N_CORES):
        o = res.results[c]["out"]  # [P, PD[4]] fp16, dim-major
        s4 = o.reshape(P, PD[4] // P, P).transpose(2, 1, 0).reshape(BPC, PD[4])
        outs.append(s4[:, : DIMS[4]].astype(np.float32))
    return np.concatenate(outs, axis=0), res


def kernel(**inputs):
    out, _ = run(inputs, trace=False)
    return out
